# revision 6
# baseline (speedup 1.0000x reference)
"""DSA indexer kernel for Trainium2 (8 NeuronCores, SPMD + AllGather).

scores[t, s] = causal_mask( sum_h w[t,h] * relu(q~[t,h] . k~[s]) * D^-0.5 )

where q~ = RoPE(q_latent @ Wq), k~ = RoPE(LN(x @ Wk)), w = x @ Wwt.
The reference also applies a Hadamard transform to both q~ and k~; it is
orthogonal and therefore preserved under the dot product, so it is skipped.

Sharding: query rows are stride-8 interleaved across the 8 cores so each
core runs an identical program (uniform causal block extents).  Keys are
computed on the owning core (contiguous 512-row shard) and AllGathered.
The upper-triangular -1e9 region is constant and is filled on the host.
"""

import numpy as np
import ml_dtypes

import concourse.bass as bass
import concourse.mybir as mybir
import concourse.tile as tile
from concourse import bacc
from concourse.bass_utils import run_bass_kernel_spmd
from concourse.masks import make_identity

F32 = mybir.dt.float32
BF16 = mybir.dt.bfloat16
NPBF16 = ml_dtypes.bfloat16

NCORES = 8
S = 4096          # sequence length
DMODEL = 2048
DCQ = 1536
H = 8
D = 128
R = 64
BASE = 10000.0
LN_EPS = 1e-5
NEG = -1e9
TPC = S // NCORES          # 512 t-rows per core
NTT = TPC // 128           # 4 t-tiles of 128 rows per core
KCH_X = DMODEL // 128      # 16 contraction chunks for x
KCH_Q = DCQ // 128         # 12 contraction chunks for q_latent

AluOp = mybir.AluOpType
ActFn = mybir.ActivationFunctionType

_CACHED = {}


def _build_program():
    nc = bacc.Bacc(
        "TRN2",
        target_bir_lowering=False,
        debug=False,
        enable_asserts=True,
        num_devices=NCORES,
    )

    def din(name, shape, dt):
        return nc.dram_tensor(name, list(shape), dt, kind="ExternalInput").ap()

    xk_d = din("xk", (128, KCH_X, TPC), BF16)      # xT chunks, own s-shard cols
    xw_d = din("xw", (128, KCH_X, TPC), BF16)      # xT chunks, own t-row cols
    qlt_d = din("qlt", (128, KCH_Q, TPC), BF16)    # q_latentT chunks, own t cols
    wq_d = din("wq", (128, KCH_Q, H * D), BF16)
    wk_d = din("wk", (128, KCH_X, D), BF16)
    wwt_d = din("wwt", (128, KCH_X, H), BF16)      # includes D^-0.5
    tak_d = din("tak", (128, NTT, 64), F32)        # k rope [cos|sin], natural
    tbk_d = din("tbk", (128, NTT, 64), F32)        # k rope [sin|cos], natural
    taq_d = din("taq", (64, H * TPC), BF16)        # q rope A rows (T layout, x8 heads)
    tbq_d = din("tbq", (64, H * TPC), BF16)        # q rope B rows
    perm_d = din("perm", (128, 128), BF16)         # rotate-half permutation
    gnat_d = din("gnat", (128, D), F32)            # ln_g broadcast tile
    bnat_d = din("bnat", (128, D), F32)            # ln_b broadcast tile
    out_d = nc.dram_tensor("outd", [NTT, 128, S], BF16, kind="ExternalOutput").ap()

    with tile.TileContext(nc) as tc:
        with (
            tc.tile_pool(name="const", bufs=1) as cpool,
            tc.tile_pool(name="big", bufs=1) as xpool,
            tc.tile_pool(name="work", bufs=4) as work,
            tc.tile_pool(name="stat", bufs=8) as stat,
            tc.tile_pool(name="acc", bufs=2) as accpool,
            tc.tile_pool(name="term", bufs=6) as termpool,
            tc.tile_pool(name="pp", bufs=2, space="PSUM") as ppsum,
            tc.tile_pool(name="zz", bufs=3, space="PSUM") as zpsum,
            tc.tile_pool(name="dram", bufs=1, space="DRAM") as dpool,
        ):
            # ---- constants / big loads
            def load(pool, ap, dt, tag):
                t = pool.tile(list(ap.shape), dt, tag=tag)
                nc.sync.dma_start(t[:], ap)
                return t

            wk_s = load(cpool, wk_d, BF16, "wk")
            wwt_s = load(cpool, wwt_d, BF16, "wwt")
            tak_s = load(cpool, tak_d, F32, "tak")
            tbk_s = load(cpool, tbk_d, F32, "tbk")
            taq_s = load(cpool, taq_d, BF16, "taq")
            tbq_s = load(cpool, tbq_d, BF16, "tbq")
            perm_s = load(cpool, perm_d, BF16, "perm")
            gnat_s = load(cpool, gnat_d, F32, "gnat")
            bnat_s = load(cpool, bnat_d, F32, "bnat")
            ident = cpool.tile([128, 128], F32, tag="ident")
            make_identity(nc, ident[:])

            xk_s = load(xpool, xk_d, BF16, "xk")
            xw_s = load(xpool, xw_d, BF16, "xw")
            qlt_s = load(xpool, qlt_d, BF16, "qlt")
            wq_s = load(xpool, wq_d, BF16, "wq")

            # ---- k-prep on own contiguous 512-row shard (natural layout)
            kTsh = xpool.tile([128, TPC], BF16, tag="kTsh")
            for st in range(NTT):
                ps = ppsum.tile([128, 512], F32, tag="pp", name="ps_k")[:, :128]
                for cc in range(KCH_X):
                    nc.tensor.matmul(
                        ps,
                        lhsT=xk_s[:, cc, st * 128 : (st + 1) * 128],
                        rhs=wk_s[:, cc, :],
                        start=(cc == 0),
                        stop=(cc == KCH_X - 1),
                    )
                sum1 = stat.tile([128, 1], F32, tag="st")
                nc.vector.tensor_reduce(sum1[:], ps, mybir.AxisListType.X, AluOp.add)
                mu = stat.tile([128, 1], F32, tag="st")
                nc.vector.tensor_scalar_mul(mu[:], sum1[:], 1.0 / D)
                kc = work.tile([128, 128], F32, tag="kc")
                nc.vector.tensor_scalar(kc[:], ps, mu[:], None, AluOp.subtract)
                sq = work.tile([128, 128], F32, tag="sq")
                ssq = stat.tile([128, 1], F32, tag="st")
                nc.scalar.activation(sq[:], kc[:], ActFn.Square, accum_out=ssq[:])
                var = stat.tile([128, 1], F32, tag="st")
                nc.vector.tensor_scalar(
                    var[:], ssq[:], 1.0 / D, LN_EPS, AluOp.mult, AluOp.add
                )
                std = stat.tile([128, 1], F32, tag="st")
                nc.scalar.activation(std[:], var[:], ActFn.Sqrt)
                rin = stat.tile([128, 1], F32, tag="st")
                nc.vector.reciprocal(rin[:], std[:])
                kn = work.tile([128, 128], F32, tag="kn")
                nc.vector.tensor_scalar_mul(kn[:], kc[:], rin[:])
                nc.vector.tensor_tensor(kn[:], kn[:], gnat_s[:], AluOp.mult)
                nc.vector.tensor_tensor(kn[:], kn[:], bnat_s[:], AluOp.add)
                # rope (free-dim slices)
                st_rows = slice(st * 128, (st + 1) * 128)
                a = work.tile([128, 64], F32, tag="ra")
                nc.vector.tensor_tensor(a[:], kn[:, 0:64], tak_s[:, st, :], AluOp.mult)
                b = work.tile([128, 64], F32, tag="rb")
                nc.vector.tensor_tensor(b[:], kn[:, 0:64], tbk_s[:, st, :], AluOp.mult)
                kr = work.tile([128, 128], F32, tag="kr")
                nc.vector.tensor_tensor(kr[:, 0:32], a[:, 0:32], a[:, 32:64], AluOp.subtract)
                nc.vector.tensor_tensor(kr[:, 32:64], b[:, 0:32], b[:, 32:64], AluOp.add)
                nc.vector.tensor_copy(kr[:, 64:128], kn[:, 64:128])
                pst = ppsum.tile([128, 512], F32, tag="pp", name="ps_t")[:, :128]
                nc.tensor.transpose(pst, kr[:], ident[:])
                nc.any.tensor_copy(kTsh[:, st_rows], pst)

            # ---- AllGather k~T shards -> full (128, S) bf16
            agin = dpool.tile([128, TPC], BF16)
            agout = dpool.tile([NCORES * 128, TPC], BF16)
            nc.sync.dma_start(agin[:], kTsh[:])
            nc.gpsimd.collective_compute(
                "AllGather",
                AluOp.bypass,
                replica_groups=[list(range(NCORES))],
                ins=[agin[:].opt()],
                outs=[agout[:].opt()],
            )
            kT = xpool.tile([128, NCORES, TPC], BF16, tag="kT")
            nc.sync.dma_start(
                kT[:], agout[:].rearrange("(r d) s -> d r s", d=128)
            )

            # ---- w-proj -> natural (t, h) layout, f32
            w_all = xpool.tile([128, NTT, H], F32, tag="w_all")
            for q4 in range(NTT):
                psw = ppsum.tile([128, 512], F32, tag="pp", name="ps_w")[:, :H]
                for cc in range(KCH_X):
                    nc.tensor.matmul(
                        psw,
                        lhsT=xw_s[:, cc, q4 * 128 : (q4 + 1) * 128],
                        rhs=wwt_s[:, cc, :],
                        start=(cc == 0),
                        stop=(cc == KCH_X - 1),
                    )
                nc.vector.tensor_copy(w_all[:, q4, :], psw)

            # ---- q-proj -> transposed layout qT (d, h*512+t), bf16
            qT = xpool.tile([128, H * TPC], BF16, tag="qT")
            for h in range(H):
                psq = ppsum.tile([128, 512], F32, tag="pp")
                for cc in range(KCH_Q):
                    nc.tensor.matmul(
                        psq,
                        lhsT=wq_s[:, cc, h * D : (h + 1) * D],
                        rhs=qlt_s[:, cc, :],
                        start=(cc == 0),
                        stop=(cc == KCH_Q - 1),
                    )
                nc.any.tensor_copy(qT[:, h * TPC : (h + 1) * TPC], psq)

            # ---- q rope in T layout via permutation matmul
            for h in range(H):
                hsl = slice(h * TPC, (h + 1) * TPC)
                psr = ppsum.tile([128, 512], F32, tag="pp")
                nc.tensor.matmul(psr, lhsT=perm_s[:], rhs=qT[:, hsl], start=True, stop=True)
                t1 = work.tile([64, TPC], BF16, tag="t1")
                nc.vector.tensor_tensor(t1[:], qT[0:64, hsl], taq_s[:, hsl], AluOp.mult)
                t2 = work.tile([64, TPC], BF16, tag="t2")
                nc.vector.tensor_tensor(t2[:], psr[0:64, :], tbq_s[:, hsl], AluOp.mult)
                nc.vector.tensor_tensor(qT[0:64, hsl], t1[:], t2[:], AluOp.add)

            # ---- main qk loop
            blk = 0
            for j in range(NTT):
                ext = (j + 1) * 1024
                acc = accpool.tile([128, S], BF16, tag="acc")
                for h in range(H):
                    lq = qT[:, h * TPC + j * 128 : h * TPC + (j + 1) * 128]
                    wcol = w_all[:, j, h : h + 1]
                    for db in range(j + 1):
                        ps = zpsum.tile([128, 1024], F32, tag="zz")
                        nc.tensor.matmul(
                            ps[:, 0:512], lhsT=lq, rhs=kT[:, 2 * db, :],
                            start=True, stop=True,
                        )
                        nc.tensor.matmul(
                            ps[:, 512:1024], lhsT=lq, rhs=kT[:, 2 * db + 1, :],
                            start=True, stop=True,
                        )
                        term = termpool.tile([128, 1024], BF16, tag="term")
                        if blk % 10 < 7:
                            nc.scalar.activation(term[:], ps[:], ActFn.Relu)
                        else:
                            nc.vector.tensor_scalar_max(term[:], ps[:], 0.0)
                        blk += 1
                        sl = acc[:, db * 1024 : (db + 1) * 1024]
                        if h == 0:
                            nc.vector.tensor_scalar(sl, term[:], wcol, None, AluOp.mult)
                        else:
                            nc.vector.scalar_tensor_tensor(
                                sl, term[:], wcol, sl, AluOp.mult, AluOp.add
                            )
                nc.sync.dma_start(out_d[j][:, 0:ext], acc[:, 0:ext])

    nc.compile()
    return nc


def _host_inputs(x, q_latent, Wq, Wk, ln_g, ln_b, Wwt):
    """Build the 8 per-core input dicts (all layout prep / constants)."""
    f32 = np.float32
    xT = np.ascontiguousarray(x[0].T.astype(f32))            # (2048, 4096)
    qlT = np.ascontiguousarray(q_latent[0].T.astype(f32))    # (1536, 4096)

    def chunk_T(a2d, kch, cols):
        # (K, n) col-slice -> (128, kch, n) contiguous
        sl = np.ascontiguousarray(a2d[:, cols])
        return np.ascontiguousarray(
            sl.reshape(kch, 128, sl.shape[1]).transpose(1, 0, 2)
        )

    wq_r = np.ascontiguousarray(
        Wq.reshape(KCH_Q, 128, H * D).transpose(1, 0, 2)
    ).astype(NPBF16)
    wk_r = np.ascontiguousarray(
        Wk.reshape(KCH_X, 128, D).transpose(1, 0, 2)
    ).astype(NPBF16)
    wwt_r = np.ascontiguousarray(
        (Wwt * (D ** -0.5)).reshape(KCH_X, 128, H).transpose(1, 0, 2)
    ).astype(NPBF16)

    freqs = (BASE ** (-(np.arange(0, R, 2, dtype=f32) / R))).astype(f32)  # (32,)
    perm = np.zeros((128, 128), dtype=NPBF16)
    for m in range(32):
        perm[m + 32, m] = 1
        perm[m, m + 32] = 1
    gnat = np.tile(np.asarray(ln_g, f32), (128, 1))
    bnat = np.tile(np.asarray(ln_b, f32), (128, 1))

    in_maps = []
    for c in range(NCORES):
        tm = c + NCORES * np.arange(TPC)
        ssl = np.arange(TPC * c, TPC * (c + 1))

        ang_k = ssl[:, None].astype(f32) * freqs                  # (512, 32)
        cos_k, sin_k = np.cos(ang_k), np.sin(ang_k)
        tak = np.concatenate([cos_k, sin_k], axis=1).astype(f32)  # (512, 64)
        tbk = np.concatenate([sin_k, cos_k], axis=1).astype(f32)
        tak = np.ascontiguousarray(tak.reshape(NTT, 128, 64).transpose(1, 0, 2))
        tbk = np.ascontiguousarray(tbk.reshape(NTT, 128, 64).transpose(1, 0, 2))

        ang_q = tm[:, None].astype(f32) * freqs
        cosT, sinT = np.cos(ang_q).T.astype(f32), np.sin(ang_q).T.astype(f32)
        taq1 = np.concatenate([cosT, cosT], axis=0)               # (64, 512)
        tbq1 = np.concatenate([-sinT, sinT], axis=0)
        taq = np.tile(taq1, (1, H)).astype(NPBF16)                # (64, 4096)
        tbq = np.tile(tbq1, (1, H)).astype(NPBF16)

        in_maps.append({
            "xk": chunk_T(xT, KCH_X, ssl).astype(NPBF16),
            "xw": chunk_T(xT, KCH_X, tm).astype(NPBF16),
            "qlt": chunk_T(qlT, KCH_Q, tm).astype(NPBF16),
            "wq": wq_r,
            "wk": wk_r,
            "wwt": wwt_r,
            "tak": tak,
            "tbk": tbk,
            "taq": taq,
            "tbq": tbq,
            "perm": perm,
            "gnat": gnat,
            "bnat": bnat,
        })
    return in_maps


def run(inputs, **spmd_kwargs):
    """Run on HW; returns (full scores (1,S,S) f32, BassKernelResults)."""
    if "nc" not in _CACHED:
        _CACHED["nc"] = _build_program()
    nc = _CACHED["nc"]
    in_maps = _host_inputs(**inputs)
    res = run_bass_kernel_spmd(nc, in_maps, core_ids=list(range(NCORES)), **spmd_kwargs)
    out = np.full((S, S), NEG, dtype=np.float32)
    for c in range(NCORES):
        tm = c + NCORES * np.arange(TPC)
        dev = res.results[c]["outd"].reshape(TPC, S).astype(np.float32)
        out[tm] = dev
    tri = np.triu_indices(S, k=1)
    out[tri] = NEG
    return out[None], res


def kernel(**inputs):
    out, _ = run(inputs)
    return out


if __name__ == "__main__":
    import sys
    if "--build" in sys.argv:
        _build_program()
        print("BUILD OK")


# revision 11
# speedup vs baseline: 1.0610x; 1.0610x over previous
"""DSA indexer kernel for Trainium2 (8 NeuronCores, SPMD + AllGather).

scores[t, s] = causal_mask( sum_h w[t,h] * relu(q~[t,h] . k~[s]) * D^-0.5 )

where q~ = RoPE(q_latent @ Wq), k~ = RoPE(LN(x @ Wk)), w = x @ Wwt.
The reference also applies a Hadamard transform to both q~ and k~; it is
orthogonal and therefore preserved under the dot product, so it is skipped.

Sharding: query rows are stride-8 interleaved across the 8 cores so each
core runs an identical program (uniform causal block extents).  Keys are
computed on the owning core (contiguous 512-row shard) and AllGathered.
The upper-triangular -1e9 region is constant and is filled on the host.
"""

import numpy as np
import ml_dtypes

import concourse.bass as bass
import concourse.mybir as mybir
import concourse.tile as tile
from concourse import bacc
from concourse.bass_utils import run_bass_kernel_spmd
from concourse.masks import make_identity

F32 = mybir.dt.float32
BF16 = mybir.dt.bfloat16
NPBF16 = ml_dtypes.bfloat16

NCORES = 8
S = 4096          # sequence length
DMODEL = 2048
DCQ = 1536
H = 8
D = 128
R = 64
BASE = 10000.0
LN_EPS = 1e-5
NEG = -1e9
TPC = S // NCORES          # 512 t-rows per core
NTT = TPC // 128           # 4 t-tiles of 128 rows per core
KCH_X = DMODEL // 128      # 16 contraction chunks for x
KCH_Q = DCQ // 128         # 12 contraction chunks for q_latent

AluOp = mybir.AluOpType
ActFn = mybir.ActivationFunctionType

_CACHED = {}


def _build_program():
    nc = bacc.Bacc(
        "TRN2",
        target_bir_lowering=False,
        debug=False,
        enable_asserts=True,
        num_devices=NCORES,
    )

    def din(name, shape, dt):
        return nc.dram_tensor(name, list(shape), dt, kind="ExternalInput").ap()

    xk_d = din("xk", (128, KCH_X, TPC), BF16)      # xT chunks, own s-shard cols
    xw_d = din("xw", (128, KCH_X, TPC), BF16)      # xT chunks, own t-row cols
    qlt_d = din("qlt", (128, KCH_Q, TPC), BF16)    # q_latentT chunks, own t cols
    wq_d = din("wq", (128, KCH_Q, H * D), BF16)
    wk_d = din("wk", (128, KCH_X, D), BF16)
    wwt_d = din("wwt", (128, KCH_X, H), BF16)      # includes D^-0.5
    tak_d = din("tak", (128, NTT, 64), F32)        # k rope [cos|sin], natural
    tbk_d = din("tbk", (128, NTT, 64), F32)        # k rope [sin|cos], natural
    taq_d = din("taq", (64, H * TPC), BF16)        # q rope A rows (T layout, x8 heads)
    tbq_d = din("tbq", (64, H * TPC), BF16)        # q rope B rows
    perm_d = din("perm", (128, 128), BF16)         # rotate-half permutation
    idbf_d = din("idbf", (128, 128), BF16)         # identity (for diag weights)
    gnat_d = din("gnat", (128, D), F32)            # ln_g broadcast tile
    bnat_d = din("bnat", (128, D), F32)            # ln_b broadcast tile
    out_d = nc.dram_tensor("outd", [NTT, 128, S], BF16, kind="ExternalOutput").ap()

    with tile.TileContext(nc) as tc:
        with (
            tc.tile_pool(name="const", bufs=1) as cpool,
            tc.tile_pool(name="big", bufs=1) as xpool,
            tc.tile_pool(name="work", bufs=4) as work,
            tc.tile_pool(name="stat", bufs=8) as stat,
            tc.tile_pool(name="acc", bufs=2) as accpool,
            tc.tile_pool(name="term", bufs=10) as termpool,
            tc.tile_pool(name="dwp", bufs=12) as dwpool,
            tc.tile_pool(name="zz", bufs=2, space="PSUM") as zpsum,
            tc.tile_pool(name="sc", bufs=4, space="PSUM") as spsum,
            tc.tile_pool(name="dram", bufs=1, space="DRAM") as dpool,
        ):
            # ---- constants / big loads
            def load(pool, ap, dt, tag):
                t = pool.tile(list(ap.shape), dt, tag=tag)
                nc.sync.dma_start(t[:], ap)
                return t

            wk_s = load(cpool, wk_d, BF16, "wk")
            wwt_s = load(cpool, wwt_d, BF16, "wwt")
            tak_s = load(cpool, tak_d, F32, "tak")
            tbk_s = load(cpool, tbk_d, F32, "tbk")
            taq_s = load(cpool, taq_d, BF16, "taq")
            tbq_s = load(cpool, tbq_d, BF16, "tbq")
            perm_s = load(cpool, perm_d, BF16, "perm")
            idbf_s = load(cpool, idbf_d, BF16, "idbf")
            gnat_s = load(cpool, gnat_d, F32, "gnat")
            bnat_s = load(cpool, bnat_d, F32, "bnat")
            ident = cpool.tile([128, 128], F32, tag="ident")
            make_identity(nc, ident[:])

            xk_s = load(xpool, xk_d, BF16, "xk")
            xw_s = load(xpool, xw_d, BF16, "xw")
            qlt_s = load(xpool, qlt_d, BF16, "qlt")
            wq_s = load(xpool, wq_d, BF16, "wq")

            # ---- k-prep on own contiguous 512-row shard (natural layout)
            kTsh = xpool.tile([128, TPC], BF16, tag="kTsh")
            for st in range(NTT):
                ps = spsum.tile([128, 512], F32, tag="sc", name="ps_k")[:, :128]
                for cc in range(KCH_X):
                    nc.tensor.matmul(
                        ps,
                        lhsT=xk_s[:, cc, st * 128 : (st + 1) * 128],
                        rhs=wk_s[:, cc, :],
                        start=(cc == 0),
                        stop=(cc == KCH_X - 1),
                    )
                sum1 = stat.tile([128, 1], F32, tag="st")
                nc.vector.tensor_reduce(sum1[:], ps, mybir.AxisListType.X, AluOp.add)
                mu = stat.tile([128, 1], F32, tag="st")
                nc.vector.tensor_scalar_mul(mu[:], sum1[:], 1.0 / D)
                kc = work.tile([128, 128], F32, tag="kc")
                nc.vector.tensor_scalar(kc[:], ps, mu[:], None, AluOp.subtract)
                sq = work.tile([128, 128], F32, tag="sq")
                ssq = stat.tile([128, 1], F32, tag="st")
                nc.scalar.activation(sq[:], kc[:], ActFn.Square, accum_out=ssq[:])
                var = stat.tile([128, 1], F32, tag="st")
                nc.vector.tensor_scalar(
                    var[:], ssq[:], 1.0 / D, LN_EPS, AluOp.mult, AluOp.add
                )
                std = stat.tile([128, 1], F32, tag="st")
                nc.scalar.activation(std[:], var[:], ActFn.Sqrt)
                rin = stat.tile([128, 1], F32, tag="st")
                nc.vector.reciprocal(rin[:], std[:])
                kn = work.tile([128, 128], F32, tag="kn")
                nc.vector.tensor_scalar_mul(kn[:], kc[:], rin[:])
                nc.vector.tensor_tensor(kn[:], kn[:], gnat_s[:], AluOp.mult)
                nc.vector.tensor_tensor(kn[:], kn[:], bnat_s[:], AluOp.add)
                # rope (free-dim slices)
                st_rows = slice(st * 128, (st + 1) * 128)
                a = work.tile([128, 64], F32, tag="ra")
                nc.vector.tensor_tensor(a[:], kn[:, 0:64], tak_s[:, st, :], AluOp.mult)
                b = work.tile([128, 64], F32, tag="rb")
                nc.vector.tensor_tensor(b[:], kn[:, 0:64], tbk_s[:, st, :], AluOp.mult)
                kr = work.tile([128, 128], F32, tag="kr")
                nc.vector.tensor_tensor(kr[:, 0:32], a[:, 0:32], a[:, 32:64], AluOp.subtract)
                nc.vector.tensor_tensor(kr[:, 32:64], b[:, 0:32], b[:, 32:64], AluOp.add)
                nc.vector.tensor_copy(kr[:, 64:128], kn[:, 64:128])
                pst = spsum.tile([128, 512], F32, tag="sc", name="ps_t")[:, :128]
                nc.tensor.transpose(pst, kr[:], ident[:])
                nc.any.tensor_copy(kTsh[:, st_rows], pst)

            # ---- AllGather k~T shards -> full (128, S) bf16
            agin = dpool.tile([128, TPC], BF16)
            agout = dpool.tile([NCORES * 128, TPC], BF16)
            nc.sync.dma_start(agin[:], kTsh[:])
            nc.gpsimd.collective_compute(
                "AllGather",
                AluOp.bypass,
                replica_groups=[list(range(NCORES))],
                ins=[agin[:].opt()],
                outs=[agout[:].opt()],
            )
            kT = xpool.tile([128, NCORES, TPC], BF16, tag="kT")
            nc.sync.dma_start(
                kT[:], agout[:].rearrange("(r d) s -> d r s", d=128)
            )

            # ---- w-proj -> natural (t, h) layout, f32
            w_all = xpool.tile([128, NTT, H], F32, tag="w_all")
            for q4 in range(NTT):
                psw = spsum.tile([128, 512], F32, tag="sc", name="ps_w")[:, :H]
                for cc in range(KCH_X):
                    nc.tensor.matmul(
                        psw,
                        lhsT=xw_s[:, cc, q4 * 128 : (q4 + 1) * 128],
                        rhs=wwt_s[:, cc, :],
                        start=(cc == 0),
                        stop=(cc == KCH_X - 1),
                    )
                nc.vector.tensor_copy(w_all[:, q4, :], psw)

            # ---- q-proj -> transposed layout qT (d, h*512+t), bf16
            qT = xpool.tile([128, H * TPC], BF16, tag="qT")
            for h in range(H):
                psq = spsum.tile([128, 512], F32, tag="sc", name="psq")
                for cc in range(KCH_Q):
                    nc.tensor.matmul(
                        psq,
                        lhsT=wq_s[:, cc, h * D : (h + 1) * D],
                        rhs=qlt_s[:, cc, :],
                        start=(cc == 0),
                        stop=(cc == KCH_Q - 1),
                    )
                nc.scalar.copy(qT[:, h * TPC : (h + 1) * TPC], psq)

            # ---- q rope in T layout via permutation matmul
            for h in range(H):
                hsl = slice(h * TPC, (h + 1) * TPC)
                psr = spsum.tile([128, 512], F32, tag="sc", name="psr")
                nc.tensor.matmul(psr, lhsT=perm_s[:], rhs=qT[:, hsl], start=True, stop=True)
                t1 = work.tile([64, TPC], BF16, tag="t1")
                nc.vector.tensor_tensor(t1[:], qT[0:64, hsl], taq_s[:, hsl], AluOp.mult)
                t2 = work.tile([64, TPC], BF16, tag="t2")
                nc.vector.tensor_tensor(t2[:], psr[0:64, :], tbq_s[:, hsl], AluOp.mult)
                nc.vector.tensor_tensor(qT[0:64, hsl], t1[:], t2[:], AluOp.add)

            # ---- main qk loop
            # Weighted head-sum: heads 0..NPE-1 via PE diag(w) matmuls
            # accumulating in PSUM; heads NPE..7 via DVE STT, seeded by the
            # PSUM drain.  All relus on ACT except every 7th on DVE.
            NPE = 6
            blk = 0
            for j in range(NTT):
                ext = (j + 1) * 1024
                acc = accpool.tile([128, S], BF16, tag="acc")
                # per-(j, h) diagonal weight tiles for the PE heads
                dws = []
                for h in range(NPE):
                    dw = dwpool.tile([128, 128], BF16, tag="dw")
                    nc.vector.tensor_scalar_mul(dw[:], idbf_s[:], w_all[:, j, h : h + 1])
                    dws.append(dw)
                for db in range(j + 1):
                    terms = []
                    for h in range(H):
                        lq = qT[:, h * TPC + j * 128 : h * TPC + (j + 1) * 128]
                        ps = zpsum.tile([128, 1024], F32, tag="zz", name="zps")
                        nc.tensor.matmul(
                            ps[:, 0:512], lhsT=lq, rhs=kT[:, 2 * db, :],
                            start=True, stop=True,
                        )
                        nc.tensor.matmul(
                            ps[:, 512:1024], lhsT=lq, rhs=kT[:, 2 * db + 1, :],
                            start=True, stop=True,
                        )
                        term = termpool.tile([128, 1024], BF16, tag="term")
                        if blk % 7 < 6:
                            nc.scalar.activation(term[:], ps[:], ActFn.Relu)
                        else:
                            nc.vector.tensor_scalar_max(term[:], ps[:], 0.0)
                        blk += 1
                        terms.append(term)
                    for half in range(2):
                        csl = slice(half * 512, (half + 1) * 512)
                        sc = spsum.tile([128, 512], F32, tag="sc", name="scps")
                        for i in range(NPE):
                            nc.tensor.matmul(
                                sc[:], lhsT=dws[i][:], rhs=terms[i][:, csl],
                                start=(i == 0), stop=(i == NPE - 1),
                            )
                        sl = acc[:, db * 1024 + half * 512 : db * 1024 + (half + 1) * 512]
                        nc.vector.tensor_copy(sl, sc[:])
                        for h in range(NPE, H):
                            nc.vector.scalar_tensor_tensor(
                                sl, terms[h][:, csl], w_all[:, j, h : h + 1], sl,
                                AluOp.mult, AluOp.add,
                            )
                nc.sync.dma_start(out_d[j][:, 0:ext], acc[:, 0:ext])

    nc.compile()
    return nc


def _host_inputs(x, q_latent, Wq, Wk, ln_g, ln_b, Wwt):
    """Build the 8 per-core input dicts (all layout prep / constants)."""
    f32 = np.float32
    xT = np.ascontiguousarray(x[0].T.astype(f32))            # (2048, 4096)
    qlT = np.ascontiguousarray(q_latent[0].T.astype(f32))    # (1536, 4096)

    def chunk_T(a2d, kch, cols):
        # (K, n) col-slice -> (128, kch, n) contiguous
        sl = np.ascontiguousarray(a2d[:, cols])
        return np.ascontiguousarray(
            sl.reshape(kch, 128, sl.shape[1]).transpose(1, 0, 2)
        )

    wq_r = np.ascontiguousarray(
        Wq.reshape(KCH_Q, 128, H * D).transpose(1, 0, 2)
    ).astype(NPBF16)
    wk_r = np.ascontiguousarray(
        Wk.reshape(KCH_X, 128, D).transpose(1, 0, 2)
    ).astype(NPBF16)
    wwt_r = np.ascontiguousarray(
        (Wwt * (D ** -0.5)).reshape(KCH_X, 128, H).transpose(1, 0, 2)
    ).astype(NPBF16)

    freqs = (BASE ** (-(np.arange(0, R, 2, dtype=f32) / R))).astype(f32)  # (32,)
    perm = np.zeros((128, 128), dtype=NPBF16)
    for m in range(32):
        perm[m + 32, m] = 1
        perm[m, m + 32] = 1
    idbf = np.eye(128, dtype=NPBF16)
    gnat = np.tile(np.asarray(ln_g, f32), (128, 1))
    bnat = np.tile(np.asarray(ln_b, f32), (128, 1))

    in_maps = []
    for c in range(NCORES):
        tm = c + NCORES * np.arange(TPC)
        ssl = np.arange(TPC * c, TPC * (c + 1))

        ang_k = ssl[:, None].astype(f32) * freqs                  # (512, 32)
        cos_k, sin_k = np.cos(ang_k), np.sin(ang_k)
        tak = np.concatenate([cos_k, sin_k], axis=1).astype(f32)  # (512, 64)
        tbk = np.concatenate([sin_k, cos_k], axis=1).astype(f32)
        tak = np.ascontiguousarray(tak.reshape(NTT, 128, 64).transpose(1, 0, 2))
        tbk = np.ascontiguousarray(tbk.reshape(NTT, 128, 64).transpose(1, 0, 2))

        ang_q = tm[:, None].astype(f32) * freqs
        cosT, sinT = np.cos(ang_q).T.astype(f32), np.sin(ang_q).T.astype(f32)
        taq1 = np.concatenate([cosT, cosT], axis=0)               # (64, 512)
        tbq1 = np.concatenate([-sinT, sinT], axis=0)
        taq = np.tile(taq1, (1, H)).astype(NPBF16)                # (64, 4096)
        tbq = np.tile(tbq1, (1, H)).astype(NPBF16)

        in_maps.append({
            "xk": chunk_T(xT, KCH_X, ssl).astype(NPBF16),
            "xw": chunk_T(xT, KCH_X, tm).astype(NPBF16),
            "qlt": chunk_T(qlT, KCH_Q, tm).astype(NPBF16),
            "wq": wq_r,
            "wk": wk_r,
            "wwt": wwt_r,
            "tak": tak,
            "tbk": tbk,
            "taq": taq,
            "tbq": tbq,
            "perm": perm,
            "idbf": idbf,
            "gnat": gnat,
            "bnat": bnat,
        })
    return in_maps


def run(inputs, **spmd_kwargs):
    """Run on HW; returns (full scores (1,S,S) f32, BassKernelResults)."""
    if "nc" not in _CACHED:
        _CACHED["nc"] = _build_program()
    nc = _CACHED["nc"]
    in_maps = _host_inputs(**inputs)
    res = run_bass_kernel_spmd(nc, in_maps, core_ids=list(range(NCORES)), **spmd_kwargs)
    out = np.full((S, S), NEG, dtype=np.float32)
    for c in range(NCORES):
        tm = c + NCORES * np.arange(TPC)
        dev = res.results[c]["outd"].reshape(TPC, S).astype(np.float32)
        out[tm] = dev
    tri = np.triu_indices(S, k=1)
    out[tri] = NEG
    return out[None], res


def kernel(**inputs):
    out, _ = run(inputs)
    return out


if __name__ == "__main__":
    import sys
    if "--build" in sys.argv:
        _build_program()
        print("BUILD OK")


# revision 13
# speedup vs baseline: 1.0777x; 1.0158x over previous
"""DSA indexer kernel for Trainium2 (8 NeuronCores, SPMD + AllGather).

scores[t, s] = causal_mask( sum_h w[t,h] * relu(q~[t,h] . k~[s]) * D^-0.5 )

where q~ = RoPE(q_latent @ Wq), k~ = RoPE(LN(x @ Wk)), w = x @ Wwt.
The reference also applies a Hadamard transform to both q~ and k~; it is
orthogonal and therefore preserved under the dot product, so it is skipped.

Sharding: query rows are stride-8 interleaved across the 8 cores so each
core runs an identical program (uniform causal block extents).  Keys are
computed on the owning core (contiguous 512-row shard) and AllGathered.
The upper-triangular -1e9 region is constant and is filled on the host.
"""

import numpy as np
import ml_dtypes

import concourse.bass as bass
import concourse.mybir as mybir
import concourse.tile as tile
from concourse import bacc
from concourse.bass_utils import run_bass_kernel_spmd
from concourse.masks import make_identity

F32 = mybir.dt.float32
BF16 = mybir.dt.bfloat16
NPBF16 = ml_dtypes.bfloat16

NCORES = 8
S = 4096          # sequence length
DMODEL = 2048
DCQ = 1536
H = 8
D = 128
R = 64
BASE = 10000.0
LN_EPS = 1e-5
NEG = -1e9
TPC = S // NCORES          # 512 t-rows per core
NTT = TPC // 128           # 4 t-tiles of 128 rows per core
KCH_X = DMODEL // 128      # 16 contraction chunks for x
KCH_Q = DCQ // 128         # 12 contraction chunks for q_latent

AluOp = mybir.AluOpType
ActFn = mybir.ActivationFunctionType

_CACHED = {}


def _build_program():
    nc = bacc.Bacc(
        "TRN2",
        target_bir_lowering=False,
        debug=False,
        enable_asserts=True,
        num_devices=NCORES,
    )

    def din(name, shape, dt):
        return nc.dram_tensor(name, list(shape), dt, kind="ExternalInput").ap()

    xk_d = din("xk", (128, KCH_X, TPC), BF16)      # xT chunks, own s-shard cols
    xw_d = din("xw", (128, KCH_X, TPC), BF16)      # xT chunks, own t-row cols
    qlt_d = din("qlt", (128, KCH_Q, TPC), BF16)    # q_latentT chunks, own t cols
    wq_d = din("wq", (128, KCH_Q, H * D), BF16)
    wk_d = din("wk", (128, KCH_X, D), BF16)
    wwt_d = din("wwt", (128, KCH_X, H), BF16)      # includes D^-0.5
    tak_d = din("tak", (128, NTT, 64), F32)        # k rope [cos|sin], natural
    tbk_d = din("tbk", (128, NTT, 64), F32)        # k rope [sin|cos], natural
    taq_d = din("taq", (64, H * TPC), BF16)        # q rope A rows (T layout, x8 heads)
    tbq_d = din("tbq", (64, H * TPC), BF16)        # q rope B rows
    perm_d = din("perm", (128, 128), BF16)         # rotate-half permutation
    idbf_d = din("idbf", (128, 128), BF16)         # identity (for diag weights)
    gnat_d = din("gnat", (128, D), F32)            # ln_g broadcast tile
    bnat_d = din("bnat", (128, D), F32)            # ln_b broadcast tile
    out_d = nc.dram_tensor("outd", [NTT, 128, S], BF16, kind="ExternalOutput").ap()

    with tile.TileContext(nc) as tc:
        with (
            tc.tile_pool(name="const", bufs=1) as cpool,
            tc.tile_pool(name="big", bufs=1) as xpool,
            tc.tile_pool(name="work", bufs=4) as work,
            tc.tile_pool(name="stat", bufs=8) as stat,
            tc.tile_pool(name="acc", bufs=2) as accpool,
            tc.tile_pool(name="term", bufs=10) as termpool,
            tc.tile_pool(name="dwp", bufs=12) as dwpool,
            tc.tile_pool(name="zz", bufs=2, space="PSUM") as zpsum,
            tc.tile_pool(name="sc", bufs=2, space="PSUM") as spsum,
            tc.tile_pool(name="kk", bufs=2, space="PSUM") as kpsum,
            tc.tile_pool(name="dram", bufs=1, space="DRAM") as dpool,
        ):
            # ---- constants / big loads
            def load(pool, ap, dt, tag):
                t = pool.tile(list(ap.shape), dt, tag=tag)
                nc.sync.dma_start(t[:], ap)
                return t

            wk_s = load(cpool, wk_d, BF16, "wk")
            wwt_s = load(cpool, wwt_d, BF16, "wwt")
            tak_s = load(cpool, tak_d, F32, "tak")
            tbk_s = load(cpool, tbk_d, F32, "tbk")
            taq_s = load(cpool, taq_d, BF16, "taq")
            tbq_s = load(cpool, tbq_d, BF16, "tbq")
            perm_s = load(cpool, perm_d, BF16, "perm")
            idbf_s = load(cpool, idbf_d, BF16, "idbf")
            gnat_s = load(cpool, gnat_d, F32, "gnat")
            bnat_s = load(cpool, bnat_d, F32, "bnat")
            ident = cpool.tile([128, 128], F32, tag="ident")
            make_identity(nc, ident[:])

            xk_s = load(xpool, xk_d, BF16, "xk")

            # ---- k-prep on own contiguous 512-row shard (natural layout)
            kTsh = xpool.tile([128, TPC], BF16, tag="kTsh")
            for st in range(NTT):
                ps = kpsum.tile([128, 512], F32, tag="kk", name="ps_k")[:, :128]
                for cc in range(KCH_X):
                    nc.tensor.matmul(
                        ps,
                        lhsT=xk_s[:, cc, st * 128 : (st + 1) * 128],
                        rhs=wk_s[:, cc, :],
                        start=(cc == 0),
                        stop=(cc == KCH_X - 1),
                    )
                sum1 = stat.tile([128, 1], F32, tag="st")
                nc.vector.tensor_reduce(sum1[:], ps, mybir.AxisListType.X, AluOp.add)
                mu = stat.tile([128, 1], F32, tag="st")
                nc.vector.tensor_scalar_mul(mu[:], sum1[:], 1.0 / D)
                kc = work.tile([128, 128], F32, tag="kc")
                nc.vector.tensor_scalar(kc[:], ps, mu[:], None, AluOp.subtract)
                sq = work.tile([128, 128], F32, tag="sq")
                ssq = stat.tile([128, 1], F32, tag="st")
                nc.scalar.activation(sq[:], kc[:], ActFn.Square, accum_out=ssq[:])
                var = stat.tile([128, 1], F32, tag="st")
                nc.vector.tensor_scalar(
                    var[:], ssq[:], 1.0 / D, LN_EPS, AluOp.mult, AluOp.add
                )
                std = stat.tile([128, 1], F32, tag="st")
                nc.scalar.activation(std[:], var[:], ActFn.Sqrt)
                rin = stat.tile([128, 1], F32, tag="st")
                nc.vector.reciprocal(rin[:], std[:])
                kn = work.tile([128, 128], F32, tag="kn")
                nc.vector.tensor_scalar_mul(kn[:], kc[:], rin[:])
                nc.vector.tensor_tensor(kn[:], kn[:], gnat_s[:], AluOp.mult)
                nc.vector.tensor_tensor(kn[:], kn[:], bnat_s[:], AluOp.add)
                # rope (free-dim slices)
                st_rows = slice(st * 128, (st + 1) * 128)
                a = work.tile([128, 64], F32, tag="ra")
                nc.vector.tensor_tensor(a[:], kn[:, 0:64], tak_s[:, st, :], AluOp.mult)
                b = work.tile([128, 64], F32, tag="rb")
                nc.vector.tensor_tensor(b[:], kn[:, 0:64], tbk_s[:, st, :], AluOp.mult)
                kr = work.tile([128, 128], F32, tag="kr")
                nc.vector.tensor_tensor(kr[:, 0:32], a[:, 0:32], a[:, 32:64], AluOp.subtract)
                nc.vector.tensor_tensor(kr[:, 32:64], b[:, 0:32], b[:, 32:64], AluOp.add)
                nc.vector.tensor_copy(kr[:, 64:128], kn[:, 64:128])
                pst = kpsum.tile([128, 512], F32, tag="kk", name="ps_t")[:, :128]
                nc.tensor.transpose(pst, kr[:], ident[:])
                nc.any.tensor_copy(kTsh[:, st_rows], pst)

            # ---- AllGather k~T shards -> full (128, S) bf16
            agin = dpool.tile([128, TPC], BF16)
            agout = dpool.tile([NCORES * 128, TPC], BF16)
            nc.sync.dma_start(agin[:], kTsh[:])
            nc.gpsimd.collective_compute(
                "AllGather",
                AluOp.bypass,
                replica_groups=[list(range(NCORES))],
                ins=[agin[:].opt()],
                outs=[agout[:].opt()],
            )
            kT = xpool.tile([128, NCORES, TPC], BF16, tag="kT")
            nc.sync.dma_start(
                kT[:], agout[:].rearrange("(r d) s -> d r s", d=128)
            )

            # big loads for the q/w side — issued after the AG trigger so the
            # collective overlaps them
            xw_s = load(xpool, xw_d, BF16, "xw")
            qlt_s = load(xpool, qlt_d, BF16, "qlt")
            wq_s = load(xpool, wq_d, BF16, "wq")

            # ---- w-proj -> natural (t, h) layout, f32
            w_all = xpool.tile([128, NTT, H], F32, tag="w_all")
            for q4 in range(NTT):
                psw = spsum.tile([128, 512], F32, tag="sc", name="ps_w")[:, :H]
                for cc in range(KCH_X):
                    nc.tensor.matmul(
                        psw,
                        lhsT=xw_s[:, cc, q4 * 128 : (q4 + 1) * 128],
                        rhs=wwt_s[:, cc, :],
                        start=(cc == 0),
                        stop=(cc == KCH_X - 1),
                    )
                nc.vector.tensor_copy(w_all[:, q4, :], psw)

            # ---- q-proj -> transposed layout qT (d, h*512+t), bf16
            qT = xpool.tile([128, H * TPC], BF16, tag="qT")
            for h in range(H):
                psq = spsum.tile([128, 512], F32, tag="sc", name="psq")
                for cc in range(KCH_Q):
                    nc.tensor.matmul(
                        psq,
                        lhsT=wq_s[:, cc, h * D : (h + 1) * D],
                        rhs=qlt_s[:, cc, :],
                        start=(cc == 0),
                        stop=(cc == KCH_Q - 1),
                    )
                nc.scalar.copy(qT[:, h * TPC : (h + 1) * TPC], psq)

            # ---- q rope in T layout via permutation matmul
            for h in range(H):
                hsl = slice(h * TPC, (h + 1) * TPC)
                psr = spsum.tile([128, 512], F32, tag="sc", name="psr")
                nc.tensor.matmul(psr, lhsT=perm_s[:], rhs=qT[:, hsl], start=True, stop=True)
                t1 = work.tile([64, TPC], BF16, tag="t1")
                nc.gpsimd.tensor_tensor(t1[:], qT[0:64, hsl], taq_s[:, hsl], AluOp.mult)
                t2 = work.tile([64, TPC], BF16, tag="t2")
                nc.vector.tensor_tensor(t2[:], psr[0:64, :], tbq_s[:, hsl], AluOp.mult)
                nc.vector.tensor_tensor(qT[0:64, hsl], t1[:], t2[:], AluOp.add)

            # ---- main qk loop
            # Heads 0..6: PE diag(w_h) matmuls accumulate the weighted sum in
            # PSUM; head 7 fuses the PSUM drain with its own MAC on DVE:
            # acc = term7*w7 + sc_psum.  Relu split ~5/7 ACT, 2/7 DVE.
            NPE = 7
            blk = 0
            for j in range(NTT):
                ext = (j + 1) * 1024
                acc = accpool.tile([128, S], BF16, tag="acc")
                dws = []
                for h in range(NPE):
                    dw = dwpool.tile([128, 128], BF16, tag="dw")
                    nc.vector.tensor_scalar_mul(dw[:], idbf_s[:], w_all[:, j, h : h + 1])
                    dws.append(dw)
                for db in range(j + 1):
                    terms = []
                    for h in range(H):
                        lq = qT[:, h * TPC + j * 128 : h * TPC + (j + 1) * 128]
                        ps = zpsum.tile([128, 1024], F32, tag="zz", name="zps")
                        nc.tensor.matmul(
                            ps[:, 0:512], lhsT=lq, rhs=kT[:, 2 * db, :],
                            start=True, stop=True,
                        )
                        nc.tensor.matmul(
                            ps[:, 512:1024], lhsT=lq, rhs=kT[:, 2 * db + 1, :],
                            start=True, stop=True,
                        )
                        term = termpool.tile([128, 1024], BF16, tag="term")
                        if blk % 7 < 5:
                            nc.scalar.activation(term[:], ps[:], ActFn.Relu)
                        else:
                            nc.vector.tensor_scalar_max(term[:], ps[:], 0.0)
                        blk += 1
                        terms.append(term)
                    for half in range(2):
                        csl = slice(half * 512, (half + 1) * 512)
                        sc = spsum.tile([128, 512], F32, tag="sc", name="scps")
                        for i in range(NPE):
                            nc.tensor.matmul(
                                sc[:], lhsT=dws[i][:], rhs=terms[i][:, csl],
                                start=(i == 0), stop=(i == NPE - 1),
                            )
                        sl = acc[:, db * 1024 + half * 512 : db * 1024 + (half + 1) * 512]
                        nc.vector.scalar_tensor_tensor(
                            sl, terms[NPE][:, csl], w_all[:, j, NPE : NPE + 1], sc[:],
                            AluOp.mult, AluOp.add,
                        )
                nc.sync.dma_start(out_d[j][:, 0:ext], acc[:, 0:ext])

    nc.compile()
    return nc


def _host_inputs(x, q_latent, Wq, Wk, ln_g, ln_b, Wwt):
    """Build the 8 per-core input dicts (all layout prep / constants)."""
    f32 = np.float32
    xT = np.ascontiguousarray(x[0].T.astype(f32))            # (2048, 4096)
    qlT = np.ascontiguousarray(q_latent[0].T.astype(f32))    # (1536, 4096)

    def chunk_T(a2d, kch, cols):
        # (K, n) col-slice -> (128, kch, n) contiguous
        sl = np.ascontiguousarray(a2d[:, cols])
        return np.ascontiguousarray(
            sl.reshape(kch, 128, sl.shape[1]).transpose(1, 0, 2)
        )

    wq_r = np.ascontiguousarray(
        Wq.reshape(KCH_Q, 128, H * D).transpose(1, 0, 2)
    ).astype(NPBF16)
    wk_r = np.ascontiguousarray(
        Wk.reshape(KCH_X, 128, D).transpose(1, 0, 2)
    ).astype(NPBF16)
    wwt_r = np.ascontiguousarray(
        (Wwt * (D ** -0.5)).reshape(KCH_X, 128, H).transpose(1, 0, 2)
    ).astype(NPBF16)

    freqs = (BASE ** (-(np.arange(0, R, 2, dtype=f32) / R))).astype(f32)  # (32,)
    perm = np.zeros((128, 128), dtype=NPBF16)
    for m in range(32):
        perm[m + 32, m] = 1
        perm[m, m + 32] = 1
    idbf = np.eye(128, dtype=NPBF16)
    gnat = np.tile(np.asarray(ln_g, f32), (128, 1))
    bnat = np.tile(np.asarray(ln_b, f32), (128, 1))

    in_maps = []
    for c in range(NCORES):
        tm = c + NCORES * np.arange(TPC)
        ssl = np.arange(TPC * c, TPC * (c + 1))

        ang_k = ssl[:, None].astype(f32) * freqs                  # (512, 32)
        cos_k, sin_k = np.cos(ang_k), np.sin(ang_k)
        tak = np.concatenate([cos_k, sin_k], axis=1).astype(f32)  # (512, 64)
        tbk = np.concatenate([sin_k, cos_k], axis=1).astype(f32)
        tak = np.ascontiguousarray(tak.reshape(NTT, 128, 64).transpose(1, 0, 2))
        tbk = np.ascontiguousarray(tbk.reshape(NTT, 128, 64).transpose(1, 0, 2))

        ang_q = tm[:, None].astype(f32) * freqs
        cosT, sinT = np.cos(ang_q).T.astype(f32), np.sin(ang_q).T.astype(f32)
        taq1 = np.concatenate([cosT, cosT], axis=0)               # (64, 512)
        tbq1 = np.concatenate([-sinT, sinT], axis=0)
        taq = np.tile(taq1, (1, H)).astype(NPBF16)                # (64, 4096)
        tbq = np.tile(tbq1, (1, H)).astype(NPBF16)

        in_maps.append({
            "xk": chunk_T(xT, KCH_X, ssl).astype(NPBF16),
            "xw": chunk_T(xT, KCH_X, tm).astype(NPBF16),
            "qlt": chunk_T(qlT, KCH_Q, tm).astype(NPBF16),
            "wq": wq_r,
            "wk": wk_r,
            "wwt": wwt_r,
            "tak": tak,
            "tbk": tbk,
            "taq": taq,
            "tbq": tbq,
            "perm": perm,
            "idbf": idbf,
            "gnat": gnat,
            "bnat": bnat,
        })
    return in_maps


def run(inputs, **spmd_kwargs):
    """Run on HW; returns (full scores (1,S,S) f32, BassKernelResults)."""
    if "nc" not in _CACHED:
        _CACHED["nc"] = _build_program()
    nc = _CACHED["nc"]
    in_maps = _host_inputs(**inputs)
    res = run_bass_kernel_spmd(nc, in_maps, core_ids=list(range(NCORES)), **spmd_kwargs)
    out = np.full((S, S), NEG, dtype=np.float32)
    for c in range(NCORES):
        tm = c + NCORES * np.arange(TPC)
        dev = res.results[c]["outd"].reshape(TPC, S).astype(np.float32)
        out[tm] = dev
    tri = np.triu_indices(S, k=1)
    out[tri] = NEG
    return out[None], res


def kernel(**inputs):
    out, _ = run(inputs)
    return out


if __name__ == "__main__":
    import sys
    if "--build" in sys.argv:
        _build_program()
        print("BUILD OK")


# revision 14
# speedup vs baseline: 1.1662x; 1.0821x over previous
"""DSA indexer kernel for Trainium2 (8 NeuronCores, SPMD + AllGather).

scores[t, s] = causal_mask( sum_h w[t,h] * relu(q~[t,h] . k~[s]) * D^-0.5 )

where q~ = RoPE(q_latent @ Wq), k~ = RoPE(LN(x @ Wk)), w = x @ Wwt.
The reference also applies a Hadamard transform to both q~ and k~; it is
orthogonal and therefore preserved under the dot product, so it is skipped.

Sharding: query rows are stride-8 interleaved across the 8 cores so each
core runs an identical program (uniform causal block extents).  Keys are
computed on the owning core (contiguous 512-row shard) and AllGathered.
The upper-triangular -1e9 region is constant and is filled on the host.
"""

import numpy as np
import ml_dtypes

import concourse.bass as bass
import concourse.mybir as mybir
import concourse.tile as tile
from concourse import bacc
from concourse.bass_utils import run_bass_kernel_spmd
from concourse.masks import make_identity

F32 = mybir.dt.float32
BF16 = mybir.dt.bfloat16
NPBF16 = ml_dtypes.bfloat16

NCORES = 8
S = 4096          # sequence length
DMODEL = 2048
DCQ = 1536
H = 8
D = 128
R = 64
BASE = 10000.0
LN_EPS = 1e-5
NEG = -1e9
TPC = S // NCORES          # 512 t-rows per core
NTT = TPC // 128           # 4 t-tiles of 128 rows per core
KCH_X = DMODEL // 128      # 16 contraction chunks for x
KCH_Q = DCQ // 128         # 12 contraction chunks for q_latent

AluOp = mybir.AluOpType
ActFn = mybir.ActivationFunctionType

_CACHED = {}


def _build_program():
    nc = bacc.Bacc(
        "TRN2",
        target_bir_lowering=False,
        debug=False,
        enable_asserts=True,
        num_devices=NCORES,
    )

    def din(name, shape, dt):
        return nc.dram_tensor(name, list(shape), dt, kind="ExternalInput").ap()

    xk_d = din("xk", (128, KCH_X, TPC), BF16)      # xT chunks, own s-shard cols
    xw_d = din("xw", (128, KCH_X, TPC), BF16)      # xT chunks, own t-row cols
    qlt_d = din("qlt", (128, KCH_Q, TPC), BF16)    # q_latentT chunks, own t cols
    wq_d = din("wq", (128, KCH_Q, H * D), BF16)
    wk_d = din("wk", (128, KCH_X, D), BF16)
    wwt_d = din("wwt", (128, KCH_X, H), BF16)      # includes D^-0.5
    tak_d = din("tak", (128, NTT, 64), F32)        # k rope [cos|sin], natural
    tbk_d = din("tbk", (128, NTT, 64), F32)        # k rope [sin|cos], natural
    taq_d = din("taq", (64, H * TPC), BF16)        # q rope A rows (T layout, x8 heads)
    tbq_d = din("tbq", (64, H * TPC), BF16)        # q rope B rows
    perm_d = din("perm", (128, 128), BF16)         # rotate-half permutation
    idbf_d = din("idbf", (128, 128), BF16)         # identity (for diag weights)
    gnat_d = din("gnat", (128, D), F32)            # ln_g broadcast tile
    bnat_d = din("bnat", (128, D), F32)            # ln_b broadcast tile
    out_d = nc.dram_tensor("outd", [NTT, 128, S], BF16, kind="ExternalOutput").ap()

    with tile.TileContext(nc) as tc:
        with (
            tc.tile_pool(name="const", bufs=1) as cpool,
            tc.tile_pool(name="big", bufs=1) as xpool,
            tc.tile_pool(name="work", bufs=4) as work,
            tc.tile_pool(name="stat", bufs=8) as stat,
            tc.tile_pool(name="acc", bufs=2) as accpool,
            tc.tile_pool(name="term", bufs=10) as termpool,
            tc.tile_pool(name="dwp", bufs=12) as dwpool,
            tc.tile_pool(name="zz", bufs=3, space="PSUM") as zpsum,
            tc.tile_pool(name="sc", bufs=2, space="PSUM") as spsum,
            tc.tile_pool(name="dram", bufs=1, space="DRAM") as dpool,
        ):
            # ---- constants / big loads
            def load(pool, ap, dt, tag):
                t = pool.tile(list(ap.shape), dt, tag=tag)
                nc.sync.dma_start(t[:], ap)
                return t

            wk_s = load(cpool, wk_d, BF16, "wk")
            wwt_s = load(cpool, wwt_d, BF16, "wwt")
            tak_s = load(cpool, tak_d, F32, "tak")
            tbk_s = load(cpool, tbk_d, F32, "tbk")
            taq_s = load(cpool, taq_d, BF16, "taq")
            tbq_s = load(cpool, tbq_d, BF16, "tbq")
            perm_s = load(cpool, perm_d, BF16, "perm")
            idbf_s = load(cpool, idbf_d, BF16, "idbf")
            gnat_s = load(cpool, gnat_d, F32, "gnat")
            bnat_s = load(cpool, bnat_d, F32, "bnat")
            ident = cpool.tile([128, 128], F32, tag="ident")
            make_identity(nc, ident[:])

            xk_s = load(xpool, xk_d, BF16, "xk")
            def load_act(pool, ap, dt, tag):
                t = pool.tile(list(ap.shape), dt, tag=tag)
                nc.scalar.dma_start(t[:], ap)
                return t
            xw_s = load_act(xpool, xw_d, BF16, "xw")
            qlt_s = load_act(xpool, qlt_d, BF16, "qlt")
            wq_s = load_act(xpool, wq_d, BF16, "wq")

            # ---- k-prep on own contiguous 512-row shard (natural layout)
            # op-major across the 4 s-tiles so the serial LN chains overlap
            kTsh = xpool.tile([128, TPC], BF16, tag="kTsh")
            kps, kcs, kns, krs, stats = [], [], [], [], []
            for st in range(NTT):
                ps = zpsum.tile([128, 1024], F32, tag="zz", name="ps_k")[:, :128]
                for cc in range(KCH_X):
                    nc.tensor.matmul(
                        ps,
                        lhsT=xk_s[:, cc, st * 128 : (st + 1) * 128],
                        rhs=wk_s[:, cc, :],
                        start=(cc == 0),
                        stop=(cc == KCH_X - 1),
                    )
                kps.append(ps)
            for st in range(NTT):
                sum1 = stat.tile([128, 1], F32, tag="st")
                nc.vector.tensor_reduce(sum1[:], kps[st], mybir.AxisListType.X, AluOp.add)
                mu = stat.tile([128, 1], F32, tag="st")
                nc.vector.tensor_scalar_mul(mu[:], sum1[:], 1.0 / D)
                kc = work.tile([128, 128], F32, tag=f"kc{st}")
                nc.vector.tensor_scalar(kc[:], kps[st], mu[:], None, AluOp.subtract)
                kcs.append(kc)
            for st in range(NTT):
                sq = work.tile([128, 128], F32, tag="sq")
                ssq = stat.tile([128, 1], F32, tag="st")
                nc.scalar.activation(sq[:], kcs[st][:], ActFn.Square, accum_out=ssq[:])
                stats.append(ssq)
            for st in range(NTT):
                var = stat.tile([128, 1], F32, tag="st")
                nc.vector.tensor_scalar(
                    var[:], stats[st][:], 1.0 / D, LN_EPS, AluOp.mult, AluOp.add
                )
                stats[st] = var
            for st in range(NTT):
                std = stat.tile([128, 1], F32, tag="st")
                nc.scalar.activation(std[:], stats[st][:], ActFn.Sqrt)
                stats[st] = std
            for st in range(NTT):
                rin = stat.tile([128, 1], F32, tag="st")
                nc.vector.reciprocal(rin[:], stats[st][:])
                stats[st] = rin
            for st in range(NTT):
                kn = work.tile([128, 128], F32, tag=f"kn{st}")
                nc.vector.tensor_scalar_mul(kn[:], kcs[st][:], stats[st][:])
                nc.vector.tensor_tensor(kn[:], kn[:], gnat_s[:], AluOp.mult)
                nc.vector.tensor_tensor(kn[:], kn[:], bnat_s[:], AluOp.add)
                kns.append(kn)
            for st in range(NTT):
                a = work.tile([128, 64], F32, tag="ra")
                nc.vector.tensor_tensor(a[:], kns[st][:, 0:64], tak_s[:, st, :], AluOp.mult)
                b = work.tile([128, 64], F32, tag="rb")
                nc.vector.tensor_tensor(b[:], kns[st][:, 0:64], tbk_s[:, st, :], AluOp.mult)
                kr = work.tile([128, 128], F32, tag=f"kr{st}")
                nc.vector.tensor_tensor(kr[:, 0:32], a[:, 0:32], a[:, 32:64], AluOp.subtract)
                nc.vector.tensor_tensor(kr[:, 32:64], b[:, 0:32], b[:, 32:64], AluOp.add)
                nc.vector.tensor_copy(kr[:, 64:128], kns[st][:, 64:128])
                krs.append(kr)
            for st in range(NTT):
                pst = spsum.tile([128, 512], F32, tag="sc", name="ps_t")[:, :128]
                nc.tensor.transpose(pst, krs[st][:], ident[:])
                nc.any.tensor_copy(kTsh[:, st * 128 : (st + 1) * 128], pst)

            # ---- AllGather k~T shards -> full (128, S) bf16
            agin = dpool.tile([128, TPC], BF16)
            agout = dpool.tile([NCORES * 128, TPC], BF16, addr_space="Shared")
            nc.sync.dma_start(agin[:], kTsh[:])
            nc.gpsimd.collective_compute(
                "AllGather",
                AluOp.bypass,
                replica_groups=[list(range(NCORES))],
                ins=[agin[:].opt()],
                outs=[agout[:].opt()],
            )
            kT = xpool.tile([128, NCORES, TPC], BF16, tag="kT")
            nc.sync.dma_start(
                kT[:], agout[:].rearrange("(r d) s -> d r s", d=128)
            )

            # ---- w-proj -> natural (t, h) layout, f32
            w_all = xpool.tile([128, NTT, H], F32, tag="w_all")
            for q4 in range(NTT):
                psw = spsum.tile([128, 512], F32, tag="sc", name="ps_w")[:, :H]
                for cc in range(KCH_X):
                    nc.tensor.matmul(
                        psw,
                        lhsT=xw_s[:, cc, q4 * 128 : (q4 + 1) * 128],
                        rhs=wwt_s[:, cc, :],
                        start=(cc == 0),
                        stop=(cc == KCH_X - 1),
                    )
                nc.vector.tensor_copy(w_all[:, q4, :], psw)

            # ---- q-proj -> transposed layout qT (d, h*512+t), bf16
            qT = xpool.tile([128, H * TPC], BF16, tag="qT")
            for h in range(H):
                psq = spsum.tile([128, 512], F32, tag="sc", name="psq")
                for cc in range(KCH_Q):
                    nc.tensor.matmul(
                        psq,
                        lhsT=wq_s[:, cc, h * D : (h + 1) * D],
                        rhs=qlt_s[:, cc, :],
                        start=(cc == 0),
                        stop=(cc == KCH_Q - 1),
                    )
                nc.scalar.copy(qT[:, h * TPC : (h + 1) * TPC], psq)

            # ---- q rope in T layout via permutation matmul
            for h in range(H):
                hsl = slice(h * TPC, (h + 1) * TPC)
                psr = spsum.tile([128, 512], F32, tag="sc", name="psr")
                nc.tensor.matmul(psr, lhsT=perm_s[:], rhs=qT[:, hsl], start=True, stop=True)
                t1 = work.tile([64, TPC], BF16, tag="t1")
                nc.gpsimd.tensor_tensor(t1[:], qT[0:64, hsl], taq_s[:, hsl], AluOp.mult)
                t2 = work.tile([64, TPC], BF16, tag="t2")
                nc.vector.tensor_tensor(t2[:], psr[0:64, :], tbq_s[:, hsl], AluOp.mult)
                nc.vector.tensor_tensor(qT[0:64, hsl], t1[:], t2[:], AluOp.add)

            # ---- main qk loop
            # Heads 0..6: PE diag(w_h) matmuls accumulate the weighted sum in
            # PSUM; head 7 fuses the PSUM drain with its own MAC on DVE:
            # acc = term7*w7 + sc_psum.  Relu split ~5/7 ACT, 2/7 DVE.
            NPE = 7
            blk = 0
            for j in range(NTT):
                ext = (j + 1) * 1024
                acc = accpool.tile([128, S], BF16, tag="acc")
                dws = []
                for h in range(NPE):
                    dw = dwpool.tile([128, 128], BF16, tag="dw")
                    nc.vector.tensor_scalar_mul(dw[:], idbf_s[:], w_all[:, j, h : h + 1])
                    dws.append(dw)
                for db in range(j + 1):
                    terms = []
                    for h in range(H):
                        lq = qT[:, h * TPC + j * 128 : h * TPC + (j + 1) * 128]
                        ps = zpsum.tile([128, 1024], F32, tag="zz", name="zps")
                        nc.tensor.matmul(
                            ps[:, 0:512], lhsT=lq, rhs=kT[:, 2 * db, :],
                            start=True, stop=True,
                        )
                        nc.tensor.matmul(
                            ps[:, 512:1024], lhsT=lq, rhs=kT[:, 2 * db + 1, :],
                            start=True, stop=True,
                        )
                        term = termpool.tile([128, 1024], BF16, tag="term")
                        if h in (5, 7):
                            nc.vector.tensor_scalar_max(term[:], ps[:], 0.0)
                        else:
                            nc.scalar.activation(term[:], ps[:], ActFn.Relu)
                        terms.append(term)
                    for half in range(2):
                        csl = slice(half * 512, (half + 1) * 512)
                        sc = spsum.tile([128, 512], F32, tag="sc", name="scps")
                        for i in range(NPE):
                            nc.tensor.matmul(
                                sc[:], lhsT=dws[i][:], rhs=terms[i][:, csl],
                                start=(i == 0), stop=(i == NPE - 1),
                            )
                        sl = acc[:, db * 1024 + half * 512 : db * 1024 + (half + 1) * 512]
                        nc.vector.scalar_tensor_tensor(
                            sl, terms[NPE][:, csl], w_all[:, j, NPE : NPE + 1], sc[:],
                            AluOp.mult, AluOp.add,
                        )
                nc.sync.dma_start(out_d[j][:, 0:ext], acc[:, 0:ext])

    nc.compile()
    return nc


def _host_inputs(x, q_latent, Wq, Wk, ln_g, ln_b, Wwt):
    """Build the 8 per-core input dicts (all layout prep / constants)."""
    f32 = np.float32
    xT = np.ascontiguousarray(x[0].T.astype(f32))            # (2048, 4096)
    qlT = np.ascontiguousarray(q_latent[0].T.astype(f32))    # (1536, 4096)

    def chunk_T(a2d, kch, cols):
        # (K, n) col-slice -> (128, kch, n) contiguous
        sl = np.ascontiguousarray(a2d[:, cols])
        return np.ascontiguousarray(
            sl.reshape(kch, 128, sl.shape[1]).transpose(1, 0, 2)
        )

    wq_r = np.ascontiguousarray(
        Wq.reshape(KCH_Q, 128, H * D).transpose(1, 0, 2)
    ).astype(NPBF16)
    wk_r = np.ascontiguousarray(
        Wk.reshape(KCH_X, 128, D).transpose(1, 0, 2)
    ).astype(NPBF16)
    wwt_r = np.ascontiguousarray(
        (Wwt * (D ** -0.5)).reshape(KCH_X, 128, H).transpose(1, 0, 2)
    ).astype(NPBF16)

    freqs = (BASE ** (-(np.arange(0, R, 2, dtype=f32) / R))).astype(f32)  # (32,)
    perm = np.zeros((128, 128), dtype=NPBF16)
    for m in range(32):
        perm[m + 32, m] = 1
        perm[m, m + 32] = 1
    idbf = np.eye(128, dtype=NPBF16)
    gnat = np.tile(np.asarray(ln_g, f32), (128, 1))
    bnat = np.tile(np.asarray(ln_b, f32), (128, 1))

    in_maps = []
    for c in range(NCORES):
        tm = c + NCORES * np.arange(TPC)
        ssl = np.arange(TPC * c, TPC * (c + 1))

        ang_k = ssl[:, None].astype(f32) * freqs                  # (512, 32)
        cos_k, sin_k = np.cos(ang_k), np.sin(ang_k)
        tak = np.concatenate([cos_k, sin_k], axis=1).astype(f32)  # (512, 64)
        tbk = np.concatenate([sin_k, cos_k], axis=1).astype(f32)
        tak = np.ascontiguousarray(tak.reshape(NTT, 128, 64).transpose(1, 0, 2))
        tbk = np.ascontiguousarray(tbk.reshape(NTT, 128, 64).transpose(1, 0, 2))

        ang_q = tm[:, None].astype(f32) * freqs
        cosT, sinT = np.cos(ang_q).T.astype(f32), np.sin(ang_q).T.astype(f32)
        taq1 = np.concatenate([cosT, cosT], axis=0)               # (64, 512)
        tbq1 = np.concatenate([-sinT, sinT], axis=0)
        taq = np.tile(taq1, (1, H)).astype(NPBF16)                # (64, 4096)
        tbq = np.tile(tbq1, (1, H)).astype(NPBF16)

        in_maps.append({
            "xk": chunk_T(xT, KCH_X, ssl).astype(NPBF16),
            "xw": chunk_T(xT, KCH_X, tm).astype(NPBF16),
            "qlt": chunk_T(qlT, KCH_Q, tm).astype(NPBF16),
            "wq": wq_r,
            "wk": wk_r,
            "wwt": wwt_r,
            "tak": tak,
            "tbk": tbk,
            "taq": taq,
            "tbq": tbq,
            "perm": perm,
            "idbf": idbf,
            "gnat": gnat,
            "bnat": bnat,
        })
    return in_maps


def run(inputs, **spmd_kwargs):
    """Run on HW; returns (full scores (1,S,S) f32, BassKernelResults)."""
    if "nc" not in _CACHED:
        _CACHED["nc"] = _build_program()
    nc = _CACHED["nc"]
    in_maps = _host_inputs(**inputs)
    res = run_bass_kernel_spmd(nc, in_maps, core_ids=list(range(NCORES)), **spmd_kwargs)
    out = np.full((S, S), NEG, dtype=np.float32)
    for c in range(NCORES):
        tm = c + NCORES * np.arange(TPC)
        dev = res.results[c]["outd"].reshape(TPC, S).astype(np.float32)
        out[tm] = dev
    tri = np.triu_indices(S, k=1)
    out[tri] = NEG
    return out[None], res


def kernel(**inputs):
    out, _ = run(inputs)
    return out


if __name__ == "__main__":
    import sys
    if "--build" in sys.argv:
        _build_program()
        print("BUILD OK")
